# revision 25
# baseline (speedup 1.0000x reference)
"""GAT (3 independent 1-head GATConvs with edge embeddings, mean over heads)
on 8 Trainium2 NeuronCores via Bass/Tile.

Strategy (edge partition by destination node, per the sharding hint: each
shard holds its edge slice + gathered features):
  * Host: sort edges by dst, split dst-node range into 8 equal contiguous
    shards (12500 nodes each) -> outputs are disjoint, no collectives.
  * Host folds W_lin into the per-head weights:  W'_h = W_lin @ W_head[h],
    v'_h = W'_h @ att_src[h], u'_h = W'_h @ att_dst[h]; so the device never
    materializes x @ W_lin.
  * Host pre-gathers per-edge feature streams into one packed fp16 input,
    one 64KB block per 128-edge tile (single DMA per 3-tile group):
      td[t] = [ x[src_e].T | x[dst_e].T | emb_table[ew_e] | one-hot(dst) ]
              (cols 0:128    128:256      256:384 edges/p   384:512)
  * Device per 128-edge tile: xh = x_src @ W' (one matmul, all heads),
    logits l = x_src.v' + x_dst.u' (two tiny matmuls into shared PSUM),
    leaky-relu + exp, msg = exp * (ew * xh), then attention-weighted
    scatter-add via one-hot matmul accumulated in PSUM over a 128-node
    window.  Softmax normalization happens once per node at window flush:
    out = sum_h agg_h / (3*(s_h + 1e-16)), s_h aggregated in the same
    matmuls as an extra 3 columns of exp values.
  * No segment-max subtraction: logits are tiny (|l| < 1.5), exp is exact
    enough in fp16, and alpha = exp(l)/sum(exp(l)) is invariant to shifts.

The program is identical on all 8 cores (SPMD); only the data differs.
"""

import os
import sys

import numpy as np

if "/opt/trn_rl_repo" not in sys.path:
    sys.path.insert(0, "/opt/trn_rl_repo")

N_NODES = 100000
D = 128
H = 3
N_CORES = 8
WIN = 128  # nodes per aggregation window (one PSUM partition block)

F16 = np.float16

# Stash of the last run's profile results (filled when BASSGAT_TRACE=1).
LAST_RESULT = {}


def _fold_weights(W_lin, emb_table, W_head, att_src, att_dst):
    Wp = np.stack([W_lin @ W_head[h] for h in range(H)])  # [H,128,128]
    vp = np.stack([Wp[h] @ att_src[h] for h in range(H)])  # [H,128]
    up = np.stack([Wp[h] @ att_dst[h] for h in range(H)])  # [H,128]
    # WV[din, 0:384] = [W'_0 | W'_1 | W'_2]; 384:387 = v'; 387:390 = u'
    WV = np.zeros((D, 390), np.float32)
    for h in range(H):
        WV[:, h * D:(h + 1) * D] = Wp[h]
        WV[:, 384 + h] = vp[h]
        WV[:, 387 + h] = up[h]
    return WV.astype(F16)


def _prepare(x, src, dst, ew_idx, emb_table, npc, n_cores):
    """Sort by dst, shard by dst range, pad each 128-node window's edges to
    tiles of 128, and build the per-core device input arrays."""
    nw = (npc + WIN - 1) // WIN  # windows per core
    order = np.argsort(dst, kind="stable")
    srcp, dstp, ewp = src[order], dst[order], ew_idx[order]

    x16 = x.astype(F16)
    emb16 = emb_table.astype(F16)

    # per-edge window id: windows are 128-node blocks RELATIVE to each
    # core's node base (c*npc), which need not be 128-aligned
    core_of_edge = dstp // npc
    rem = dstp - core_of_edge * npc
    win_of_edge = core_of_edge * nw + rem // WIN  # [E]
    n_win_total = n_cores * nw
    cnt = np.bincount(win_of_edge, minlength=n_win_total)  # edges per window
    tiles_per_win = (cnt + 127) // 128
    T = int(max(1, tiles_per_win.max()))  # uniform tiles/window (padded)
    nt = nw * T  # tiles per core

    # position of each edge inside the padded layout
    win_start = np.zeros(n_win_total + 1, np.int64)
    np.cumsum(cnt, out=win_start[1:])
    idx_in_win = np.arange(len(dstp)) - win_start[win_of_edge]
    slot = (win_of_edge * (T * 128)) + idx_in_win  # global padded slot

    n_slots = n_win_total * T * 128
    srcpad = np.zeros(n_slots, np.int64)
    ewpad = np.zeros(n_slots, np.int64)
    ldpad = np.full(n_slots, 255, np.int64)  # 255 => one-hot row of zeros
    xdpad = np.zeros(n_slots, np.int64)
    srcpad[slot] = srcp
    ewpad[slot] = ewp
    xdpad[slot] = dstp
    ldpad[slot] = rem % WIN

    per_core = []
    csl = nt * 128  # slots per core
    onehot_cols = np.arange(WIN)
    for c in range(n_cores):
        sl = slice(c * csl, (c + 1) * csl)
        td = np.empty((nt, 128, 512), F16)
        td[:, :, 0:128] = x16[srcpad[sl]].reshape(nt, 128, D).transpose(0, 2, 1)
        td[:, :, 128:256] = x16[xdpad[sl]].reshape(nt, 128, D).transpose(0, 2, 1)
        td[:, :, 256:384] = emb16[ewpad[sl]].reshape(nt, 128, D)
        td[:, :, 384:512] = (
            ldpad[sl].reshape(nt, 128, 1) == onehot_cols[None, None, :]
        )
        per_core.append({"td": td})
    return per_core, T, nw


def _build_program(nt, nw, T, npc, reps=1):
    import concourse.bass as bass
    from concourse import mybir
    from concourse.tile import TileContext

    f16 = mybir.dt.float16
    f32 = mybir.dt.float32
    AF = mybir.ActivationFunctionType
    OP = mybir.AluOpType

    nc = bass.Bass(trn_type="TRN2", target_bir_lowering=False)

    TD = nc.dram_tensor("td", [nt, 128, 512], f16, kind="ExternalInput")
    WV = nc.dram_tensor("wv", [128, 390], f16, kind="ExternalInput")
    OUT = nc.dram_tensor("out", [npc, 128], f32, kind="ExternalOutput")

    with TileContext(nc) as tc:
        with (
            tc.tile_pool(name="wv", bufs=1) as wvp,
            tc.tile_pool(name="td", bufs=10) as tdp,
            tc.tile_pool(name="msg", bufs=6) as msgp,
            tc.tile_pool(name="exp", bufs=6) as expp,
            tc.tile_pool(name="lrelu", bufs=6) as lrp,
            tc.tile_pool(name="fl", bufs=4) as flp,
            tc.tile_pool(name="acc", bufs=4) as accp,
            tc.tile_pool(name="proj", bufs=4, space="PSUM") as projp,
            tc.tile_pool(name="lps", bufs=2, space="PSUM") as lpsp,
            tc.tile_pool(name="agg", bufs=2, space="PSUM") as aggp,
        ):
            wv_sb = wvp.tile([128, 390], f16)
            nc.sync.dma_start(out=wv_sb[:], in_=WV[:])

            for w in [w_ for _ in range(reps) for w_ in range(nw)]:
                rows = min(WIN, npc - w * WIN)
                agg_ps = aggp.tile([128, 387], f32)
                acc_sb = accp.tile([128, 128], f32)

                for g0 in range(0, T, 3):
                    gt = min(3, T - g0)  # tiles in this group
                    l_ps = lpsp.tile([128, 9], f32)
                    exp_sb = expp.tile([128, 9], f32, tag="exp32")
                    lrelu_sb = lrp.tile([128, 9], f32)
                    msg_sb = msgp.tile([128, 3 * 390], f16)
                    projs = []
                    ohs = []
                    tdg_sb = tdp.tile([128, 3 * 512], f16)
                    g_lo = w * T + g0
                    nc.sync.dma_start(
                        out=tdg_sb[:, 0:512 * gt],
                        in_=TD[g_lo:g_lo + gt].rearrange("t p f -> p t f"),
                    )
                    for t3 in range(gt):
                        t = g0 + t3
                        td_sb = tdg_sb[:, 512 * t3:512 * (t3 + 1)]
                        xx_sb, ew_sb, oh_sb = td_sb, td_sb, td_sb

                        proj_ps = projp.tile([128, 384], f32)
                        # xh for all 3 heads: [e, 384]
                        nc.tensor.matmul(
                            proj_ps[:], lhsT=xx_sb[:, 0:128],
                            rhs=wv_sb[:, 0:384], start=True, stop=True,
                        )
                        # logits: a_src + a_dst accumulated in l_ps cols
                        nc.tensor.matmul(
                            l_ps[:, 3 * t3:3 * t3 + 3], lhsT=xx_sb[:, 0:128],
                            rhs=wv_sb[:, 384:387], start=True, stop=False,
                        )
                        nc.tensor.matmul(
                            l_ps[:, 3 * t3:3 * t3 + 3], lhsT=xx_sb[:, 128:256],
                            rhs=wv_sb[:, 387:390], start=False, stop=True,
                        )
                        projs.append(proj_ps)
                        ohs.append((oh_sb, ew_sb))

                    # exp(leaky_relu(l)) = max(exp(l), exp(0.2*l)) since exp
                    # is monotonic (the HW Lrelu ignores its alpha operand)
                    nfree = 3 * gt
                    nc.scalar.activation(
                        out=lrelu_sb[:, 0:nfree], in_=l_ps[:, 0:nfree],
                        func=AF.Exp, scale=0.2,
                    )
                    nc.scalar.activation(
                        out=exp_sb[:, 0:nfree], in_=l_ps[:, 0:nfree],
                        func=AF.Exp,
                    )
                    nc.vector.tensor_max(
                        exp_sb[:, 0:nfree], exp_sb[:, 0:nfree],
                        lrelu_sb[:, 0:nfree],
                    )

                    for t3 in range(gt):
                        t = g0 + t3
                        proj_ps = projs[t3]
                        oh_sb, ew_sb = ohs[t3]
                        B = 390 * t3
                        # msg_h = exp_h * ew * xh_h
                        # h0 fused on DVE: (xh0 * exp0) * ew
                        nc.vector.scalar_tensor_tensor(
                            out=msg_sb[:, B:B + 128], in0=proj_ps[:, 0:128],
                            scalar=exp_sb[:, 3 * t3:3 * t3 + 1], in1=ew_sb[:, 256:384],
                            op0=OP.mult, op1=OP.mult,
                        )
                        # h1, h2: scaled PSUM->SBUF copy on ScalarE...
                        nc.scalar.activation(
                            out=msg_sb[:, B + 128:B + 256],
                            in_=proj_ps[:, 128:256], func=AF.Copy,
                            scale=exp_sb[:, 3 * t3 + 1:3 * t3 + 2],
                        )
                        nc.scalar.activation(
                            out=msg_sb[:, B + 256:B + 384],
                            in_=proj_ps[:, 256:384], func=AF.Copy,
                            scale=exp_sb[:, 3 * t3 + 2:3 * t3 + 3],
                        )
                        # ... then * ew on DVE (two blocks)
                        nc.vector.tensor_mul(
                            msg_sb[:, B + 128:B + 256],
                            msg_sb[:, B + 128:B + 256], ew_sb[:, 256:384],
                        )
                        nc.vector.tensor_mul(
                            msg_sb[:, B + 256:B + 384],
                            msg_sb[:, B + 256:B + 384], ew_sb[:, 256:384],
                        )

                    # exp (fp16) into cols 384:387 of each tile's msg block
                    # (last DVE write of the group: the agg matmul below then
                    # needs just one DVE wait)
                    nc.vector.tensor_copy(
                        msg_sb[:].rearrange("p (g c) -> p g c", g=3)[:, 0:gt, 384:387],
                        exp_sb[:].rearrange("p (g c) -> p g c", g=3)[:, 0:gt, 0:3],
                    )
                    for t3 in range(gt):
                        t = g0 + t3
                        oh_sb, _ = ohs[t3]
                        B = 390 * t3
                        # agg[n, 0:384] += oh.T @ msg ; [384:387] += oh.T @ exp
                        nc.tensor.matmul(
                            agg_ps[:, 0:387], lhsT=oh_sb[:, 384:512],
                            rhs=msg_sb[:, B:B + 387],
                            start=(t == 0), stop=(t == T - 1),
                            skip_group_check=True,
                        )

                # flush window: out[n] = sum_h agg_h[n] / (3*(s_h[n]+1e-16))
                s3_sb = flp.tile([128, 3], f32, tag="s3")
                r_sb = flp.tile([128, 3], f32, tag="r")
                tmp1 = flp.tile([128, 128], f32, tag="t1")
                tmp2 = flp.tile([128, 128], f32, tag="t2")
                nc.vector.tensor_scalar(
                    out=s3_sb[:], in0=agg_ps[:, 384:387],
                    scalar1=3.0, scalar2=3e-16, op0=OP.mult, op1=OP.add,
                )
                nc.vector.reciprocal(r_sb[:], s3_sb[:])
                nc.scalar.activation(
                    out=acc_sb[:], in_=agg_ps[:, 0:128], func=AF.Copy,
                    scale=r_sb[:, 0:1],
                )
                nc.vector.tensor_scalar(
                    out=tmp1[:], in0=agg_ps[:, 128:256],
                    scalar1=r_sb[:, 1:2], scalar2=None, op0=OP.mult,
                )
                nc.vector.tensor_scalar(
                    out=tmp2[:], in0=agg_ps[:, 256:384],
                    scalar1=r_sb[:, 2:3], scalar2=None, op0=OP.mult,
                )
                nc.gpsimd.tensor_add(acc_sb[:], acc_sb[:], tmp1[:])
                nc.gpsimd.tensor_add(acc_sb[:], acc_sb[:], tmp2[:])
                nc.sync.dma_start(
                    out=OUT[w * WIN:w * WIN + rows, :], in_=acc_sb[0:rows, :]
                )
    return nc


def _split_excess_waits(nc):
    """Walrus rejects TPB instructions carrying more than one semaphore
    wait (and DMAs with more than a few).  Hoist excess waits into
    standalone EventSemaphore (wait-only) instructions on the same engine,
    placed immediately before the instruction — semantically identical,
    since the engine stalls at the wait either way."""
    from concourse import mybir

    LIMITS = {"InstDMACopy": 1}
    SKIP = {"InstEventSemaphore", "InstCall", "InstISA",
            "InstRegisterMove", "InstUnconditionalBranch", "InstMemset"}
    ctr = 0
    for fn in nc.m.functions:
        for blk in fn.blocks:
            insts = blk.instructions
            out = []
            changed = False
            for inst in insts:
                tname = type(inst).__name__
                si = inst.sync_info
                limit = LIMITS.get(tname, 1)
                if (tname not in SKIP and si is not None and si.on_wait
                        and len(si.on_wait) > limit):
                    waits = list(si.on_wait)
                    for wt in waits[:-limit]:
                        evs = mybir.InstEventSemaphore(
                            name=f"WSPLIT-{ctr}", engine=inst.engine,
                            ins=[], outs=[],
                            sync_info=mybir.SyncInfo(
                                on_wait=[wt], on_update=[]),
                        )
                        ctr += 1
                        out.append(evs)
                    inst.sync_info = mybir.SyncInfo(
                        on_wait=waits[-limit:], on_update=list(si.on_update)
                    )
                    changed = True
                out.append(inst)
            if changed:
                blk.instructions = out
    return ctr


def _make_runner(nc, in_maps, n_cores):
    """Build a reusable jitted executable for the SPMD program (the
    multi-core body of bass2jax.run_bass_via_pjrt, kept callable so the
    bench can time steady-state executions)."""
    import jax
    from jax.sharding import Mesh, PartitionSpec
    from jax.experimental.shard_map import shard_map
    from concourse import bass2jax, mybir
    from concourse.bass2jax import _bass_exec_p, install_neuronx_cc_hook

    install_neuronx_cc_hook()
    partition_name = (
        nc.partition_id_tensor.name if nc.partition_id_tensor else None
    )
    in_names, out_names, out_avals, zero_outs = [], [], [], []
    for alloc in nc.m.functions[0].allocations:
        if not isinstance(alloc, mybir.MemoryLocationSet):
            continue
        name = alloc.memorylocations[0].name
        if alloc.kind == "ExternalInput":
            if name != partition_name:
                in_names.append(name)
        elif alloc.kind == "ExternalOutput":
            out_names.append(name)
            shape = tuple(alloc.tensor_shape)
            dtype = mybir.dt.np(alloc.dtype)
            out_avals.append(jax.core.ShapedArray(shape, dtype))
            zero_outs.append(np.zeros(shape, dtype))
    n_params = len(in_names)
    n_outs = len(out_avals)
    all_in_names = list(in_names) + list(out_names)
    if partition_name is not None:
        all_in_names.append(partition_name)
    donate = tuple(range(n_params, n_params + n_outs))

    def _body(*args):
        operands = list(args)
        if partition_name is not None:
            operands.append(bass2jax.partition_id_tensor())
        outs = _bass_exec_p.bind(
            *operands,
            out_avals=tuple(out_avals),
            in_names=tuple(all_in_names),
            out_names=tuple(out_names),
            lowering_input_output_aliases=(),
            sim_require_finite=True,
            sim_require_nnan=True,
            nc=nc,
        )
        return tuple(outs)

    devices = jax.devices()[:n_cores]
    mesh = Mesh(np.asarray(devices), ("core",))
    in_specs = (PartitionSpec("core"),) * (n_params + n_outs)
    out_specs = (PartitionSpec("core"),) * n_outs
    sharded = jax.jit(
        shard_map(_body, mesh=mesh, in_specs=in_specs, out_specs=out_specs,
                  check_rep=False),
        donate_argnums=donate, keep_unused=True,
    )
    sharding = jax.sharding.NamedSharding(mesh, PartitionSpec("core"))
    concat_in = [
        jax.device_put(
            np.concatenate([np.asarray(m[name]) for m in in_maps], axis=0),
            sharding,
        )
        for name in in_names
    ]
    zero_template = [
        (tuple([n_cores * z.shape[0], *z.shape[1:]]), z.dtype)
        for z in zero_outs
    ]
    import jax.numpy as jnp

    make_zeros = jax.jit(
        lambda: tuple(jnp.zeros(shp, dt) for shp, dt in zero_template),
        out_shardings=tuple(sharding for _ in zero_template),
    )

    def run_nosync():
        zeros = make_zeros()  # on-device, no host->device transfer
        return sharded(*concat_in, *zeros)

    def run():
        out_arrs = run_nosync()
        jax.block_until_ready(out_arrs)
        return out_arrs

    def results_of(out_arrs):
        return [
            {
                name: np.asarray(out_arrs[i]).reshape(
                    n_cores, *out_avals[i].shape)[c]
                for i, name in enumerate(out_names)
            }
            for c in range(n_cores)
        ]

    return run, run_nosync, results_of


def _run(x, src, dst, ew_idx, W_lin, emb_table, W_head, att_src, att_dst,
         bias, n_nodes, n_cores):
    import time

    npc = n_nodes // n_cores
    WV16 = _fold_weights(W_lin, emb_table, W_head, att_src, att_dst)
    per_core, T, nw = _prepare(x, src, dst, ew_idx, emb_table, npc, n_cores)
    nt = nw * T

    nc = _build_program(nt, nw, T, npc)
    _split_excess_waits(nc)

    in_maps = [{"td": pc["td"], "wv": WV16} for pc in per_core]
    run, run_nosync, results_of = _make_runner(nc, in_maps, n_cores)
    out_arrs = run()  # first call compiles

    bench_iters = int(os.environ.get("BASSGAT_BENCH", "0"))
    if bench_iters > 0:
        import jax

        times = []
        for _ in range(3):
            t0 = time.perf_counter()
            out_arrs = run()
            times.append(time.perf_counter() - t0)
        # pipelined: issue many executions back-to-back, sync once, so
        # per-call dispatch overhead is amortized away
        piped = []
        for _ in range(3):
            t0 = time.perf_counter()
            for _ in range(bench_iters):
                out_arrs = run_nosync()
            jax.block_until_ready(out_arrs)
            piped.append((time.perf_counter() - t0) / bench_iters)
        LAST_RESULT["exec_time_ns"] = int(min(min(times), min(piped)) * 1e9)
        LAST_RESULT["bench_times"] = times
        LAST_RESULT["piped_s"] = piped
    else:
        LAST_RESULT["exec_time_ns"] = None

    res = results_of(out_arrs)
    out = np.empty((n_nodes, 128), np.float32)
    for c in range(n_cores):
        out[c * npc:(c + 1) * npc] = res[c]["out"]
    out += bias.mean(axis=0)
    return out


def kernel(x, edge_index, edge_weight, W_lin, emb_table, W_head, att_src,
           att_dst, bias):
    x = np.asarray(x, np.float32)
    src = np.asarray(edge_index[0], np.int64)
    dst = np.asarray(edge_index[1], np.int64)
    ew_idx = np.asarray(edge_weight, np.int64)
    return _run(
        x, src, dst, ew_idx,
        np.asarray(W_lin, np.float32), np.asarray(emb_table, np.float32),
        np.asarray(W_head, np.float32), np.asarray(att_src, np.float32),
        np.asarray(att_dst, np.float32), np.asarray(bias, np.float32),
        N_NODES, N_CORES,
    )


# revision 26
# speedup vs baseline: 1.0645x; 1.0645x over previous
"""GAT (3 independent 1-head GATConvs with edge embeddings, mean over heads)
on 8 Trainium2 NeuronCores via Bass/Tile.

Strategy (edge partition by destination node, per the sharding hint: each
shard holds its edge slice + gathered features):
  * Host: sort edges by dst, split dst-node range into 8 equal contiguous
    shards (12500 nodes each) -> outputs are disjoint, no collectives.
  * Host folds W_lin into the per-head weights:  W'_h = W_lin @ W_head[h],
    v'_h = W'_h @ att_src[h], u'_h = W'_h @ att_dst[h]; so the device never
    materializes x @ W_lin.
  * Host pre-gathers per-edge feature streams into one packed fp16 input,
    one 64KB block per 128-edge tile (single DMA per 3-tile group):
      td[t] = [ x[src_e].T | x[dst_e].T | emb_table[ew_e] | one-hot(dst) ]
              (cols 0:128    128:256      256:384 edges/p   384:512)
  * Device per 128-edge tile: xh = x_src @ W' (one matmul, all heads),
    logits l = x_src.v' + x_dst.u' (two tiny matmuls into shared PSUM),
    leaky-relu + exp, msg = exp * (ew * xh), then attention-weighted
    scatter-add via one-hot matmul accumulated in PSUM over a 128-node
    window.  Softmax normalization happens once per node at window flush:
    out = sum_h agg_h / (3*(s_h + 1e-16)), s_h aggregated in the same
    matmuls as an extra 3 columns of exp values.
  * No segment-max subtraction: logits are tiny (|l| < 1.5), exp is exact
    enough in fp16, and alpha = exp(l)/sum(exp(l)) is invariant to shifts.

The program is identical on all 8 cores (SPMD); only the data differs.
"""

import os
import sys

import numpy as np

if "/opt/trn_rl_repo" not in sys.path:
    sys.path.insert(0, "/opt/trn_rl_repo")

N_NODES = 100000
D = 128
H = 3
N_CORES = 8
WIN = 128  # nodes per aggregation window (one PSUM partition block)

F16 = np.float16

# Stash of the last run's profile results (filled when BASSGAT_TRACE=1).
LAST_RESULT = {}


def _fold_weights(W_lin, emb_table, W_head, att_src, att_dst):
    Wp = np.stack([W_lin @ W_head[h] for h in range(H)])  # [H,128,128]
    vp = np.stack([Wp[h] @ att_src[h] for h in range(H)])  # [H,128]
    up = np.stack([Wp[h] @ att_dst[h] for h in range(H)])  # [H,128]
    # WV[din, 0:384] = [W'_0 | W'_1 | W'_2]; 384:387 = v'; 387:390 = u'
    WV = np.zeros((D, 390), np.float32)
    for h in range(H):
        WV[:, h * D:(h + 1) * D] = Wp[h]
        WV[:, 384 + h] = vp[h]
        WV[:, 387 + h] = up[h]
    return WV.astype(F16)


def _prepare(x, src, dst, ew_idx, emb_table, npc, n_cores):
    """Sort by dst, shard by dst range, pad each 128-node window's edges to
    tiles of 128, and build the per-core device input arrays."""
    nw = (npc + WIN - 1) // WIN  # windows per core
    order = np.argsort(dst, kind="stable")
    srcp, dstp, ewp = src[order], dst[order], ew_idx[order]

    x16 = x.astype(F16)
    emb16 = emb_table.astype(F16)

    # per-edge window id: windows are 128-node blocks RELATIVE to each
    # core's node base (c*npc), which need not be 128-aligned
    core_of_edge = dstp // npc
    rem = dstp - core_of_edge * npc
    win_of_edge = core_of_edge * nw + rem // WIN  # [E]
    n_win_total = n_cores * nw
    cnt = np.bincount(win_of_edge, minlength=n_win_total)  # edges per window
    tiles_per_win = (cnt + 127) // 128
    T = int(max(1, tiles_per_win.max()))  # uniform tiles/window (padded)
    nt = nw * T  # tiles per core

    # position of each edge inside the padded layout
    win_start = np.zeros(n_win_total + 1, np.int64)
    np.cumsum(cnt, out=win_start[1:])
    idx_in_win = np.arange(len(dstp)) - win_start[win_of_edge]
    slot = (win_of_edge * (T * 128)) + idx_in_win  # global padded slot

    n_slots = n_win_total * T * 128
    srcpad = np.zeros(n_slots, np.int64)
    ewpad = np.zeros(n_slots, np.int64)
    ldpad = np.full(n_slots, 255, np.int64)  # 255 => one-hot row of zeros
    xdpad = np.zeros(n_slots, np.int64)
    srcpad[slot] = srcp
    ewpad[slot] = ewp
    xdpad[slot] = dstp
    ldpad[slot] = rem % WIN

    per_core = []
    csl = nt * 128  # slots per core
    onehot_cols = np.arange(WIN)
    for c in range(n_cores):
        sl = slice(c * csl, (c + 1) * csl)
        td = np.empty((nt, 128, 512), F16)
        td[:, :, 0:128] = x16[srcpad[sl]].reshape(nt, 128, D).transpose(0, 2, 1)
        td[:, :, 128:256] = x16[xdpad[sl]].reshape(nt, 128, D).transpose(0, 2, 1)
        td[:, :, 256:384] = emb16[ewpad[sl]].reshape(nt, 128, D)
        td[:, :, 384:512] = (
            ldpad[sl].reshape(nt, 128, 1) == onehot_cols[None, None, :]
        )
        per_core.append({"td": td})
    return per_core, T, nw


def _build_program(nt, nw, T, npc, reps=1):
    import concourse.bass as bass
    from concourse import mybir
    from concourse.tile import TileContext

    f16 = mybir.dt.float16
    f32 = mybir.dt.float32
    AF = mybir.ActivationFunctionType
    OP = mybir.AluOpType

    nc = bass.Bass(trn_type="TRN2", target_bir_lowering=False)

    TD = nc.dram_tensor("td", [nt, 128, 512], f16, kind="ExternalInput")
    WV = nc.dram_tensor("wv", [128, 390], f16, kind="ExternalInput")
    OUT = nc.dram_tensor("out", [npc, 128], f32, kind="ExternalOutput")

    with TileContext(nc) as tc:
        with (
            tc.tile_pool(name="wv", bufs=1) as wvp,
            tc.tile_pool(name="td", bufs=14) as tdp,
            tc.tile_pool(name="msg", bufs=8) as msgp,
            tc.tile_pool(name="exp", bufs=6) as expp,
            tc.tile_pool(name="lrelu", bufs=6) as lrp,
            tc.tile_pool(name="fl", bufs=4) as flp,
            tc.tile_pool(name="acc", bufs=4) as accp,
            tc.tile_pool(name="proj", bufs=4, space="PSUM") as projp,
            tc.tile_pool(name="lps", bufs=2, space="PSUM") as lpsp,
            tc.tile_pool(name="agg", bufs=2, space="PSUM") as aggp,
        ):
            wv_sb = wvp.tile([128, 390], f16)
            nc.sync.dma_start(out=wv_sb[:], in_=WV[:])

            for w in [w_ for _ in range(reps) for w_ in range(nw)]:
                rows = min(WIN, npc - w * WIN)
                agg_ps = aggp.tile([128, 387], f32)
                acc_sb = accp.tile([128, 128], f32)

                for g0 in range(0, T, 3):
                    gt = min(3, T - g0)  # tiles in this group
                    l_ps = lpsp.tile([128, 9], f32)
                    exp_sb = expp.tile([128, 9], f32, tag="exp32")
                    lrelu_sb = lrp.tile([128, 9], f32)
                    msg_sb = msgp.tile([128, 3 * 390], f16)
                    projs = []
                    ohs = []
                    tdg_sb = tdp.tile([128, 3 * 512], f16)
                    g_lo = w * T + g0
                    nc.sync.dma_start(
                        out=tdg_sb[:, 0:512 * gt],
                        in_=TD[g_lo:g_lo + gt].rearrange("t p f -> p t f"),
                    )
                    for t3 in range(gt):
                        t = g0 + t3
                        td_sb = tdg_sb[:, 512 * t3:512 * (t3 + 1)]
                        xx_sb, ew_sb, oh_sb = td_sb, td_sb, td_sb

                        proj_ps = projp.tile([128, 384], f32)
                        # xh for all 3 heads: [e, 384]
                        nc.tensor.matmul(
                            proj_ps[:], lhsT=xx_sb[:, 0:128],
                            rhs=wv_sb[:, 0:384], start=True, stop=True,
                        )
                        # logits: a_src + a_dst accumulated in l_ps cols
                        nc.tensor.matmul(
                            l_ps[:, 3 * t3:3 * t3 + 3], lhsT=xx_sb[:, 0:128],
                            rhs=wv_sb[:, 384:387], start=True, stop=False,
                        )
                        nc.tensor.matmul(
                            l_ps[:, 3 * t3:3 * t3 + 3], lhsT=xx_sb[:, 128:256],
                            rhs=wv_sb[:, 387:390], start=False, stop=True,
                        )
                        projs.append(proj_ps)
                        ohs.append((oh_sb, ew_sb))

                    # exp(leaky_relu(l)) = max(exp(l), exp(0.2*l)) since exp
                    # is monotonic (the HW Lrelu ignores its alpha operand)
                    nfree = 3 * gt
                    nc.scalar.activation(
                        out=lrelu_sb[:, 0:nfree], in_=l_ps[:, 0:nfree],
                        func=AF.Exp, scale=0.2,
                    )
                    nc.scalar.activation(
                        out=exp_sb[:, 0:nfree], in_=l_ps[:, 0:nfree],
                        func=AF.Exp,
                    )
                    nc.vector.tensor_max(
                        exp_sb[:, 0:nfree], exp_sb[:, 0:nfree],
                        lrelu_sb[:, 0:nfree],
                    )

                    for t3 in range(gt):
                        t = g0 + t3
                        proj_ps = projs[t3]
                        oh_sb, ew_sb = ohs[t3]
                        B = 390 * t3
                        # msg_h = exp_h * ew * xh_h
                        # h0 fused on DVE: (xh0 * exp0) * ew
                        nc.vector.scalar_tensor_tensor(
                            out=msg_sb[:, B:B + 128], in0=proj_ps[:, 0:128],
                            scalar=exp_sb[:, 3 * t3:3 * t3 + 1], in1=ew_sb[:, 256:384],
                            op0=OP.mult, op1=OP.mult,
                        )
                        # h1, h2: scaled PSUM->SBUF copy on ScalarE...
                        nc.scalar.activation(
                            out=msg_sb[:, B + 128:B + 256],
                            in_=proj_ps[:, 128:256], func=AF.Copy,
                            scale=exp_sb[:, 3 * t3 + 1:3 * t3 + 2],
                        )
                        nc.scalar.activation(
                            out=msg_sb[:, B + 256:B + 384],
                            in_=proj_ps[:, 256:384], func=AF.Copy,
                            scale=exp_sb[:, 3 * t3 + 2:3 * t3 + 3],
                        )
                        # ... then * ew on DVE (one op, ew broadcast x2)
                        h12 = msg_sb[:, B + 128:B + 384].rearrange(
                            "p (a d) -> p a d", a=2)
                        nc.vector.tensor_mul(
                            h12, h12,
                            ew_sb[:, 256:384].unsqueeze(1).broadcast_to(
                                (128, 2, 128)),
                        )

                    # exp (fp16) into cols 384:387 of each tile's msg block
                    # (last DVE write of the group: the agg matmul below then
                    # needs just one DVE wait)
                    nc.vector.tensor_copy(
                        msg_sb[:].rearrange("p (g c) -> p g c", g=3)[:, 0:gt, 384:387],
                        exp_sb[:].rearrange("p (g c) -> p g c", g=3)[:, 0:gt, 0:3],
                    )
                    for t3 in range(gt):
                        t = g0 + t3
                        oh_sb, _ = ohs[t3]
                        B = 390 * t3
                        # agg[n, 0:384] += oh.T @ msg ; [384:387] += oh.T @ exp
                        nc.tensor.matmul(
                            agg_ps[:, 0:387], lhsT=oh_sb[:, 384:512],
                            rhs=msg_sb[:, B:B + 387],
                            start=(t == 0), stop=(t == T - 1),
                            skip_group_check=True,
                        )

                # flush window: out[n] = sum_h agg_h[n] / (3*(s_h[n]+1e-16))
                s3_sb = flp.tile([128, 3], f32, tag="s3")
                r_sb = flp.tile([128, 3], f32, tag="r")
                tmp1 = flp.tile([128, 128], f32, tag="t1")
                tmp2 = flp.tile([128, 128], f32, tag="t2")
                nc.vector.tensor_scalar(
                    out=s3_sb[:], in0=agg_ps[:, 384:387],
                    scalar1=3.0, scalar2=3e-16, op0=OP.mult, op1=OP.add,
                )
                nc.vector.reciprocal(r_sb[:], s3_sb[:])
                nc.scalar.activation(
                    out=acc_sb[:], in_=agg_ps[:, 0:128], func=AF.Copy,
                    scale=r_sb[:, 0:1],
                )
                nc.vector.tensor_scalar(
                    out=tmp1[:], in0=agg_ps[:, 128:256],
                    scalar1=r_sb[:, 1:2], scalar2=None, op0=OP.mult,
                )
                nc.vector.tensor_scalar(
                    out=tmp2[:], in0=agg_ps[:, 256:384],
                    scalar1=r_sb[:, 2:3], scalar2=None, op0=OP.mult,
                )
                nc.gpsimd.tensor_add(acc_sb[:], acc_sb[:], tmp1[:])
                nc.gpsimd.tensor_add(acc_sb[:], acc_sb[:], tmp2[:])
                nc.sync.dma_start(
                    out=OUT[w * WIN:w * WIN + rows, :], in_=acc_sb[0:rows, :]
                )
    return nc


def _split_excess_waits(nc):
    """Walrus rejects TPB instructions carrying more than one semaphore
    wait (and DMAs with more than a few).  Hoist excess waits into
    standalone EventSemaphore (wait-only) instructions on the same engine,
    placed immediately before the instruction — semantically identical,
    since the engine stalls at the wait either way."""
    from concourse import mybir

    LIMITS = {"InstDMACopy": 1}
    SKIP = {"InstEventSemaphore", "InstCall", "InstISA",
            "InstRegisterMove", "InstUnconditionalBranch", "InstMemset"}
    ctr = 0
    for fn in nc.m.functions:
        for blk in fn.blocks:
            insts = blk.instructions
            out = []
            changed = False
            for inst in insts:
                tname = type(inst).__name__
                si = inst.sync_info
                limit = LIMITS.get(tname, 1)
                if (tname not in SKIP and si is not None and si.on_wait
                        and len(si.on_wait) > limit):
                    waits = list(si.on_wait)
                    for wt in waits[:-limit]:
                        evs = mybir.InstEventSemaphore(
                            name=f"WSPLIT-{ctr}", engine=inst.engine,
                            ins=[], outs=[],
                            sync_info=mybir.SyncInfo(
                                on_wait=[wt], on_update=[]),
                        )
                        ctr += 1
                        out.append(evs)
                    inst.sync_info = mybir.SyncInfo(
                        on_wait=waits[-limit:], on_update=list(si.on_update)
                    )
                    changed = True
                out.append(inst)
            if changed:
                blk.instructions = out
    return ctr


def _make_runner(nc, in_maps, n_cores):
    """Build a reusable jitted executable for the SPMD program (the
    multi-core body of bass2jax.run_bass_via_pjrt, kept callable so the
    bench can time steady-state executions)."""
    import jax
    from jax.sharding import Mesh, PartitionSpec
    from jax.experimental.shard_map import shard_map
    from concourse import bass2jax, mybir
    from concourse.bass2jax import _bass_exec_p, install_neuronx_cc_hook

    install_neuronx_cc_hook()
    partition_name = (
        nc.partition_id_tensor.name if nc.partition_id_tensor else None
    )
    in_names, out_names, out_avals, zero_outs = [], [], [], []
    for alloc in nc.m.functions[0].allocations:
        if not isinstance(alloc, mybir.MemoryLocationSet):
            continue
        name = alloc.memorylocations[0].name
        if alloc.kind == "ExternalInput":
            if name != partition_name:
                in_names.append(name)
        elif alloc.kind == "ExternalOutput":
            out_names.append(name)
            shape = tuple(alloc.tensor_shape)
            dtype = mybir.dt.np(alloc.dtype)
            out_avals.append(jax.core.ShapedArray(shape, dtype))
            zero_outs.append(np.zeros(shape, dtype))
    n_params = len(in_names)
    n_outs = len(out_avals)
    all_in_names = list(in_names) + list(out_names)
    if partition_name is not None:
        all_in_names.append(partition_name)
    donate = tuple(range(n_params, n_params + n_outs))

    def _body(*args):
        operands = list(args)
        if partition_name is not None:
            operands.append(bass2jax.partition_id_tensor())
        outs = _bass_exec_p.bind(
            *operands,
            out_avals=tuple(out_avals),
            in_names=tuple(all_in_names),
            out_names=tuple(out_names),
            lowering_input_output_aliases=(),
            sim_require_finite=True,
            sim_require_nnan=True,
            nc=nc,
        )
        return tuple(outs)

    devices = jax.devices()[:n_cores]
    mesh = Mesh(np.asarray(devices), ("core",))
    in_specs = (PartitionSpec("core"),) * (n_params + n_outs)
    out_specs = (PartitionSpec("core"),) * n_outs
    sharded = jax.jit(
        shard_map(_body, mesh=mesh, in_specs=in_specs, out_specs=out_specs,
                  check_rep=False),
        donate_argnums=donate, keep_unused=True,
    )
    sharding = jax.sharding.NamedSharding(mesh, PartitionSpec("core"))
    concat_in = [
        jax.device_put(
            np.concatenate([np.asarray(m[name]) for m in in_maps], axis=0),
            sharding,
        )
        for name in in_names
    ]
    zero_template = [
        (tuple([n_cores * z.shape[0], *z.shape[1:]]), z.dtype)
        for z in zero_outs
    ]
    import jax.numpy as jnp

    make_zeros = jax.jit(
        lambda: tuple(jnp.zeros(shp, dt) for shp, dt in zero_template),
        out_shardings=tuple(sharding for _ in zero_template),
    )

    def run_nosync():
        zeros = make_zeros()  # on-device, no host->device transfer
        return sharded(*concat_in, *zeros)

    def run():
        out_arrs = run_nosync()
        jax.block_until_ready(out_arrs)
        return out_arrs

    def results_of(out_arrs):
        return [
            {
                name: np.asarray(out_arrs[i]).reshape(
                    n_cores, *out_avals[i].shape)[c]
                for i, name in enumerate(out_names)
            }
            for c in range(n_cores)
        ]

    return run, run_nosync, results_of


def _run(x, src, dst, ew_idx, W_lin, emb_table, W_head, att_src, att_dst,
         bias, n_nodes, n_cores):
    import time

    npc = n_nodes // n_cores
    WV16 = _fold_weights(W_lin, emb_table, W_head, att_src, att_dst)
    per_core, T, nw = _prepare(x, src, dst, ew_idx, emb_table, npc, n_cores)
    nt = nw * T

    nc = _build_program(nt, nw, T, npc)
    _split_excess_waits(nc)

    in_maps = [{"td": pc["td"], "wv": WV16} for pc in per_core]
    run, run_nosync, results_of = _make_runner(nc, in_maps, n_cores)
    out_arrs = run()  # first call compiles

    bench_iters = int(os.environ.get("BASSGAT_BENCH", "0"))
    if bench_iters > 0:
        import jax

        times = []
        for _ in range(3):
            t0 = time.perf_counter()
            out_arrs = run()
            times.append(time.perf_counter() - t0)
        # pipelined: issue many executions back-to-back, sync once, so
        # per-call dispatch overhead is amortized away
        piped = []
        for _ in range(3):
            t0 = time.perf_counter()
            for _ in range(bench_iters):
                out_arrs = run_nosync()
            jax.block_until_ready(out_arrs)
            piped.append((time.perf_counter() - t0) / bench_iters)
        LAST_RESULT["exec_time_ns"] = int(min(min(times), min(piped)) * 1e9)
        LAST_RESULT["bench_times"] = times
        LAST_RESULT["piped_s"] = piped
    else:
        LAST_RESULT["exec_time_ns"] = None

    res = results_of(out_arrs)
    out = np.empty((n_nodes, 128), np.float32)
    for c in range(n_cores):
        out[c * npc:(c + 1) * npc] = res[c]["out"]
    out += bias.mean(axis=0)
    return out


def kernel(x, edge_index, edge_weight, W_lin, emb_table, W_head, att_src,
           att_dst, bias):
    x = np.asarray(x, np.float32)
    src = np.asarray(edge_index[0], np.int64)
    dst = np.asarray(edge_index[1], np.int64)
    ew_idx = np.asarray(edge_weight, np.int64)
    return _run(
        x, src, dst, ew_idx,
        np.asarray(W_lin, np.float32), np.asarray(emb_table, np.float32),
        np.asarray(W_head, np.float32), np.asarray(att_src, np.float32),
        np.asarray(att_dst, np.float32), np.asarray(bias, np.float32),
        N_NODES, N_CORES,
    )
